# revision 39
# baseline (speedup 1.0000x reference)
"""Trainium2 Bass kernel for nn_GatedSpikingReservoirStep.

Reference computation (per batch row):
    prev = prev_state[:, :2048]
    input_part = inputs @ W_in.T                    # [B, R]
    reservoir_part = prev @ W_res.T                 # [B, R]
    gate = sigmoid(inputs @ W_gate.T)               # [B, 3R] -> i, f, o
    state = 0.9 * f * prev + 0.1 * tanh(i * (input_part + reservoir_part))
    state = o * state
    state = where(state > 0.5, state - 0.5, state)
    out = pad(state, [B, 2560])

Strategy: data-parallel over batch (8 cores x 512 rows). All matmuls are
computed transposed (out[r, b] = W_slice @ x_shard.T) so the contraction
dim (d or r') is the SBUF partition dim for both operands. The host
pre-packs every operand so each device DMA is per-partition contiguous.
Matmuls run in float32r (TF32-like multiply, fp32 accumulate, 4x the
fp32 rate). Software pipeline: the three gate GEMMs for reservoir tile
t+1 run ahead of the state GEMM for tile t, so the early tiles only
wait on x + gate weights while prev/W_in/W_res stream in; the
gate/tanh/spike epilogue runs on ACT + DVE one tile behind the PE.
"""

import numpy as np

B = 4096
D = 1024
R = 2048
MAX_DIM = 2560
N_CORES = 8
BS = B // N_CORES          # 512 batch rows per core
RT = R // 128              # 16 reservoir tiles of 128
KD = D // 128              # 8 contraction chunks over input dim
KR = R // 128              # 16 contraction chunks over reservoir dim

LEAK = 0.1
THRESH = 0.5

# 'f32r' (fast, ~1.5e-4 matmul rel err) or 'f32' (exact, 4x slower)
MM_MODE = 'f32r'

_cache = {}


def _build_nc():
    """Build and compile the per-core Bass module (same NEFF on all cores)."""
    import concourse.mybir as mybir
    import concourse.tile as tile
    from concourse import bacc

    F32 = mybir.dt.float32
    MMDT = mybir.dt.float32r if MM_MODE == 'f32r' else mybir.dt.float32
    AF = mybir.ActivationFunctionType
    OP = mybir.AluOpType

    nc = bacc.Bacc("TRN2", target_bir_lowering=False, debug=False)

    # Host-packed inputs; all are [128-partition, contiguous-free] blocks.
    x_d = nc.dram_tensor("x", [KD, 128, BS], MMDT, kind="ExternalInput")
    p_d = nc.dram_tensor("p", [KR, 128, BS], MMDT, kind="ExternalInput")
    win_d = nc.dram_tensor("win", [RT, 128, KD, 128], MMDT, kind="ExternalInput")
    wres_d = nc.dram_tensor("wres", [RT, 128, KR, 128], MMDT, kind="ExternalInput")
    wg_d = nc.dram_tensor("wg", [3, RT, 128, KD, 128], MMDT, kind="ExternalInput")
    out_d = nc.dram_tensor("out", [R, BS], F32, kind="ExternalOutput")

    with tile.TileContext(nc) as tc:
        with (
            tc.tile_pool(name="acts", bufs=1) as acts,
            tc.tile_pool(name="wpool", bufs=3) as wpool,
            tc.tile_pool(name="epi", bufs=2) as epi,
            tc.tile_pool(name="psum", bufs=2, space="PSUM") as psum,
        ):
            wg_ts = {}
            win_ts = {}
            wres_ts = {}
            x_ks = []
            p_ks = []

            def load_wg_g(t, g, split=False):
                w = wg_ts.get(t)
                if w is None:
                    w = wpool.tile([128, 3, KD, 128], MMDT, tag="wg",
                                   name=f"wg{t}", bufs=4)
                    wg_ts[t] = w
                if split:
                    h = KD // 2
                    nc.sync.dma_start(w[:, g, :h], wg_d.ap()[g, t, :, :h])
                    nc.sync.dma_start(w[:, g, h:], wg_d.ap()[g, t, :, h:])
                else:
                    nc.sync.dma_start(w[:, g], wg_d.ap()[g, t])

            def load_win(t):
                w = wpool.tile([128, KD, 128], MMDT, tag="win", name=f"win{t}",
                               bufs=4)
                win_ts[t] = w
                nc.sync.dma_start(w[:], win_d.ap()[t])

            def load_wres(t):
                w = wpool.tile([128, KR, 128], MMDT, tag="wres", name=f"wres{t}",
                               bufs=4)
                wres_ts[t] = w
                # two k-half DMAs (512KB each) so the W_res group's first
                # matmuls can start while the second half still streams
                nc.sync.dma_start(w[:, :KR // 2], wres_d.ap()[t, :, :KR // 2])
                nc.sync.dma_start(w[:, KR // 2:], wres_d.ap()[t, :, KR // 2:])

            # ---- DMA front, single queue, in order of first PE use,
            # interleaved so the cold-rate PE is never waiting.
            def load_x(k):
                xk = acts.tile([128, BS], MMDT, tag=f"x{k}", name=f"x{k}")
                nc.sync.dma_start(xk[:], x_d.ap()[k])
                x_ks.append(xk)

            FILLER_N = 256

            load_wg_g(0, 0, split=True)
            # x0 rides the otherwise-idle scalar ring so its completion
            # overlaps wg0's -- the first matmul needs exactly these two.
            x0 = acts.tile([128, BS], MMDT, tag="x0", name="x0")
            nc.scalar.dma_start(x0[:], x_d.ap()[0])
            x_ks.append(x0)
            load_x(1)
            load_x(2)
            load_wg_g(0, 1)
            load_x(3)
            load_x(4)
            load_wg_g(0, 2)
            load_x(5)
            load_x(6)
            load_wg_g(1, 0)
            load_x(7)
            load_wg_g(1, 1)
            load_wg_g(1, 2)
            load_win(0)
            load_wres(0)
            # prev splits across both HWDGE rings (scalar is idle after x0),
            # so tiles 2-4's weights on sync aren't delayed behind it
            hb = BS // 2
            for k in range(KR):
                pk = acts.tile([128, BS], MMDT, tag=f"p{k}")
                eng = nc.scalar if k % 2 == 0 else nc.sync
                eng.dma_start(pk[:], p_d.ap()[k])
                p_ks.append(pk)

            ps_gates = {}

            def gate_mms(t, fillers=0):
                wg_t = wg_ts[t]
                ps_i = psum.tile([128, BS], F32, tag="ps_i")
                ps_f = psum.tile([128, BS], F32, tag="ps_f")
                ps_o = psum.tile([128, BS], F32, tag="ps_o")
                ps_gates[t] = (ps_i, ps_f, ps_o)
                # Filler matmuls on resident data keep the PE activity
                # monitor warm through DMA-starved stretches; the real
                # start=True matmul below discards their result.
                for j in range(fillers):
                    nc.tensor.matmul(ps_i[:, :FILLER_N], x_ks[0][:, :128],
                                     x_ks[1][:, :FILLER_N],
                                     start=(j == 0), stop=(j == fillers - 1))
                # k-outer: 3 matmuls per x chunk, so the cold-rate PE never
                # outruns the arriving x stream on the first tiles (keeps
                # the HAM activity window contiguous -> warms in 3.4us)
                for k in range(KD):
                    for ps, wslice in ((ps_i, wg_t[:, 0, k]),
                                       (ps_f, wg_t[:, 1, k]),
                                       (ps_o, wg_t[:, 2, k])):
                        nc.tensor.matmul(ps[:], wslice, x_ks[k][:],
                                         start=(k == 0), stop=(k == KD - 1))

            def epilogue(t, ps_s, lo, n):
                """state = o*(0.9*f*prev + 0.1*tanh(i*s)) + spike, columns
                [lo, lo+n)."""
                ps_i, ps_f, ps_o = ps_gates[t]
                sl = np.s_[:, lo:lo + n]
                prev_t = p_ks[t][sl]
                if MM_MODE == 'f32r':
                    prev_t = prev_t.bitcast(F32)
                si = epi.tile([128, BS], F32, tag="si")
                nc.scalar.activation(si[sl], ps_i[sl], AF.Sigmoid)
                sf = epi.tile([128, BS], F32, tag="sf")
                nc.scalar.activation(sf[sl], ps_f[sl], AF.Sigmoid)
                so = epi.tile([128, BS], F32, tag="so")
                nc.scalar.activation(so[sl], ps_o[sl], AF.Sigmoid)
                fp9 = epi.tile([128, BS], F32, tag="fp9")
                nc.vector.scalar_tensor_tensor(fp9[sl], sf[sl], 1.0 - LEAK,
                                               prev_t, OP.mult, OP.mult)
                x1 = epi.tile([128, BS], F32, tag="x1")
                nc.vector.tensor_tensor(x1[sl], si[sl], ps_s[sl], OP.mult)
                th = epi.tile([128, BS], F32, tag="th")
                nc.scalar.activation(th[sl], x1[sl], AF.Tanh)
                pre = epi.tile([128, BS], F32, tag="pre")
                nc.vector.scalar_tensor_tensor(pre[sl], th[sl], LEAK, fp9[sl],
                                               OP.mult, OP.add)
                st = epi.tile([128, BS], F32, tag="st")
                nc.vector.tensor_tensor(st[sl], pre[sl], so[sl], OP.mult)
                msk = epi.tile([128, BS], F32, tag="msk")
                nc.vector.tensor_scalar(msk[sl], st[sl], THRESH, THRESH,
                                        OP.is_gt, OP.mult)
                ot = epi.tile([128, BS], F32, tag="ot")
                nc.vector.tensor_tensor(ot[sl], st[sl], msk[sl], OP.subtract)
                # stores ride the otherwise-idle SWDGE ring so they never
                # contend with the input streams; the last tile's go on the
                # (by then idle) sync ring for low latency
                dma_eng = nc.sync if t == RT - 1 else nc.gpsimd
                dma_eng.dma_start(out_d.ap()[t * 128:(t + 1) * 128, lo:lo + n],
                                  ot[sl])

            # ---- pipelined main loop: gates for t+1, then state GEMM +
            # epilogue for t.
            gate_mms(0)
            for t in range(RT):
                # prefetch loads, in next-use order
                if t + 2 < RT:
                    for g in range(3):
                        load_wg_g(t + 2, g)
                if t + 1 < RT:
                    load_win(t + 1)
                    load_wres(t + 1)

                if t + 1 < RT:
                    gate_mms(t + 1)

                # s = input_part + reservoir_part for tile t
                win_t, wres_t = win_ts.pop(t), wres_ts.pop(t)
                ps_s = psum.tile([128, BS], F32, tag="ps_s")
                if t == RT - 1:
                    # last tile: column-halved GEMM (N=256 keeps the full
                    # fp32r rate) so the left half's epilogue overlaps the
                    # right half's matmuls and the tail chain is short
                    for lo in (0, hb):
                        cs = np.s_[:, lo:lo + hb]
                        for k in range(KD):
                            nc.tensor.matmul(ps_s[cs], win_t[:, k],
                                             x_ks[k][cs],
                                             start=(k == 0), stop=False)
                        for k in range(KR):
                            nc.tensor.matmul(ps_s[cs], wres_t[:, k],
                                             p_ks[k][cs],
                                             start=False, stop=(k == KR - 1))
                        epilogue(t, ps_s, lo, hb)
                else:
                    for k in range(KD):
                        nc.tensor.matmul(ps_s[:], win_t[:, k], x_ks[k][:],
                                         start=(k == 0), stop=False)
                    for k in range(KR):
                        nc.tensor.matmul(ps_s[:], wres_t[:, k], p_ks[k][:],
                                         start=False, stop=(k == KR - 1))
                    epilogue(t, ps_s, 0, BS)
                del ps_gates[t]

    nc.compile()
    return nc


def _get_nc():
    if 'nc' not in _cache:
        _cache['nc'] = _build_nc()
    return _cache['nc']


def _pack_inputs(inputs, prev_state, W_in, W_res, W_gate):
    """Host-side packing: transpose so contraction dim lands on SBUF
    partitions, with per-partition-contiguous DMA blocks."""
    f = np.float32
    # x[c, k, p, b] = inputs[512c + b, 128k + p]
    xp = np.ascontiguousarray(
        inputs.reshape(N_CORES, BS, KD, 128).transpose(0, 2, 3, 1).astype(f, copy=False))
    # p[c, k, p, b] = prev_state[512c + b, 128k + p]
    pp = np.ascontiguousarray(
        prev_state[:, :R].reshape(N_CORES, BS, KR, 128).transpose(0, 2, 3, 1).astype(f, copy=False))
    # win[t, p, k, m] = W_in[128t + m, 128k + p]
    win = np.ascontiguousarray(
        W_in.reshape(RT, 128, KD, 128).transpose(0, 3, 2, 1).astype(f, copy=False))
    # wres[t, p, j, m] = W_res[128t + m, 128j + p]
    wres = np.ascontiguousarray(
        W_res.reshape(RT, 128, KR, 128).transpose(0, 3, 2, 1).astype(f, copy=False))
    # wg[g, t, p, k, m] = W_gate[2048g + 128t + m, 128k + p]
    wg = np.ascontiguousarray(
        W_gate.reshape(3, RT, 128, KD, 128).transpose(0, 1, 4, 3, 2).astype(f, copy=False))

    in_maps = []
    for c in range(N_CORES):
        in_maps.append({
            "x": xp[c], "p": pp[c],
            "win": win, "wres": wres, "wg": wg,
        })
    return in_maps


def _assemble(results):
    out = np.zeros((B, MAX_DIM), dtype=np.float32)
    for c in range(N_CORES):
        out[c * BS:(c + 1) * BS, :R] = results[c]["out"].T
    return out


def _run(in_maps, **spmd_kwargs):
    from concourse.bass_utils import run_bass_kernel_spmd
    nc = _get_nc()
    return run_bass_kernel_spmd(nc, in_maps, core_ids=list(range(N_CORES)),
                                **spmd_kwargs)


def _device_ok():
    """True if this process can reach the 8 trn2 NeuronCores via PJRT."""
    try:
        import jax
        return len([d for d in jax.devices() if 'NC' in str(d)
                    or d.platform not in ('cpu',)]) >= N_CORES
    except Exception:
        return False


def _kernel_subprocess(inputs, prev_state, W_in, W_res, W_gate):
    """Run in a child process with a clean JAX_PLATFORMS so the axon PJRT
    backend is reachable even if the caller pinned jax to cpu."""
    import os
    import subprocess
    import sys
    import tempfile
    tmp = tempfile.mkdtemp(prefix="gsr_kernel_")
    in_npz = os.path.join(tmp, "in.npz")
    out_npy = os.path.join(tmp, "out.npy")
    np.savez(in_npz, inputs=np.asarray(inputs),
             prev_state=np.asarray(prev_state), W_in=np.asarray(W_in),
             W_res=np.asarray(W_res), W_gate=np.asarray(W_gate))
    code = (
        "import sys, numpy as np\n"
        f"sys.path.insert(0, {os.path.dirname(os.path.abspath(__file__))!r})\n"
        "import kernel\n"
        f"d = np.load({in_npz!r})\n"
        "out = kernel._kernel_impl(**{k: d[k] for k in d.files})\n"
        f"np.save({out_npy!r}, out)\n"
    )
    env = dict(os.environ)
    env.pop("JAX_PLATFORMS", None)
    for attempt in range(3):
        r = subprocess.run([sys.executable, "-c", code], env=env)
        if r.returncode == 0:
            return np.load(out_npy)
    r.check_returncode()


def _kernel_impl(inputs, prev_state, W_in, W_res, W_gate):
    in_maps = _pack_inputs(np.asarray(inputs), np.asarray(prev_state),
                           np.asarray(W_in), np.asarray(W_res),
                           np.asarray(W_gate))
    res = _run(in_maps)
    return _assemble(res.results)


def kernel(inputs, prev_state, W_in, W_res, W_gate):
    if _device_ok():
        try:
            return _kernel_impl(inputs, prev_state, W_in, W_res, W_gate)
        except Exception:
            # transient device errors (e.g. NRT_EXEC_UNIT_UNRECOVERABLE)
            # recover in a fresh process with a fresh PJRT client
            pass
    return _kernel_subprocess(inputs, prev_state, W_in, W_res, W_gate)


# revision 40
# speedup vs baseline: 1.0650x; 1.0650x over previous
"""Trainium2 Bass kernel for nn_GatedSpikingReservoirStep.

Reference computation (per batch row):
    prev = prev_state[:, :2048]
    input_part = inputs @ W_in.T                    # [B, R]
    reservoir_part = prev @ W_res.T                 # [B, R]
    gate = sigmoid(inputs @ W_gate.T)               # [B, 3R] -> i, f, o
    state = 0.9 * f * prev + 0.1 * tanh(i * (input_part + reservoir_part))
    state = o * state
    state = where(state > 0.5, state - 0.5, state)
    out = pad(state, [B, 2560])

Strategy: data-parallel over batch (8 cores x 512 rows). All matmuls are
computed transposed (out[r, b] = W_slice @ x_shard.T) so the contraction
dim (d or r') is the SBUF partition dim for both operands. The host
pre-packs every operand so each device DMA is per-partition contiguous.
Matmuls run in float32r (TF32-like multiply, fp32 accumulate, 4x the
fp32 rate). Software pipeline: the three gate GEMMs for reservoir tile
t+1 run ahead of the state GEMM for tile t, so the early tiles only
wait on x + gate weights while prev/W_in/W_res stream in; the
gate/tanh/spike epilogue runs on ACT + DVE one tile behind the PE.
"""

import numpy as np

B = 4096
D = 1024
R = 2048
MAX_DIM = 2560
N_CORES = 8
BS = B // N_CORES          # 512 batch rows per core
RT = R // 128              # 16 reservoir tiles of 128
KD = D // 128              # 8 contraction chunks over input dim
KR = R // 128              # 16 contraction chunks over reservoir dim

LEAK = 0.1
THRESH = 0.5

# 'f32r' (fast, ~1.5e-4 matmul rel err) or 'f32' (exact, 4x slower)
MM_MODE = 'f32r'

_cache = {}


def _build_nc():
    """Build and compile the per-core Bass module (same NEFF on all cores)."""
    import concourse.mybir as mybir
    import concourse.tile as tile
    from concourse import bacc

    F32 = mybir.dt.float32
    MMDT = mybir.dt.float32r if MM_MODE == 'f32r' else mybir.dt.float32
    AF = mybir.ActivationFunctionType
    OP = mybir.AluOpType

    nc = bacc.Bacc("TRN2", target_bir_lowering=False, debug=False)

    # Host-packed inputs; all are [128-partition, contiguous-free] blocks.
    x_d = nc.dram_tensor("x", [KD, 128, BS], MMDT, kind="ExternalInput")
    p_d = nc.dram_tensor("p", [KR, 128, BS], MMDT, kind="ExternalInput")
    win_d = nc.dram_tensor("win", [RT, 128, KD, 128], MMDT, kind="ExternalInput")
    wres_d = nc.dram_tensor("wres", [RT, 128, KR, 128], MMDT, kind="ExternalInput")
    wg_d = nc.dram_tensor("wg", [3, RT, 128, KD, 128], MMDT, kind="ExternalInput")
    out_d = nc.dram_tensor("out", [R, BS], F32, kind="ExternalOutput")

    with tile.TileContext(nc) as tc:
        with (
            tc.tile_pool(name="acts", bufs=1) as acts,
            tc.tile_pool(name="wpool", bufs=3) as wpool,
            tc.tile_pool(name="epi", bufs=2) as epi,
            tc.tile_pool(name="psum", bufs=2, space="PSUM") as psum,
        ):
            wg_ts = {}
            win_ts = {}
            wres_ts = {}
            x_ks = []
            p_ks = []

            def load_wg_g(t, g, split=False):
                w = wg_ts.get(t)
                if w is None:
                    w = wpool.tile([128, 3, KD, 128], MMDT, tag="wg",
                                   name=f"wg{t}", bufs=4)
                    wg_ts[t] = w
                if split:
                    h = KD // 2
                    nc.sync.dma_start(w[:, g, :h], wg_d.ap()[g, t, :, :h])
                    nc.sync.dma_start(w[:, g, h:], wg_d.ap()[g, t, :, h:])
                else:
                    nc.sync.dma_start(w[:, g], wg_d.ap()[g, t])

            def load_win(t):
                w = wpool.tile([128, KD, 128], MMDT, tag="win", name=f"win{t}",
                               bufs=4)
                win_ts[t] = w
                nc.sync.dma_start(w[:], win_d.ap()[t])

            def load_wres(t):
                w = wpool.tile([128, KR, 128], MMDT, tag="wres", name=f"wres{t}",
                               bufs=4)
                wres_ts[t] = w
                nc.sync.dma_start(w[:], wres_d.ap()[t])

            # ---- DMA front, single queue, in order of first PE use,
            # interleaved so the cold-rate PE is never waiting.
            def load_x(k):
                xk = acts.tile([128, BS], MMDT, tag=f"x{k}", name=f"x{k}")
                nc.sync.dma_start(xk[:], x_d.ap()[k])
                x_ks.append(xk)

            FILLER_N = 256

            load_wg_g(0, 0, split=True)
            # x0 rides the otherwise-idle scalar ring so its completion
            # overlaps wg0's -- the first matmul needs exactly these two.
            x0 = acts.tile([128, BS], MMDT, tag="x0", name="x0")
            nc.scalar.dma_start(x0[:], x_d.ap()[0])
            x_ks.append(x0)
            load_x(1)
            load_x(2)
            load_wg_g(0, 1)
            load_x(3)
            load_x(4)
            load_wg_g(0, 2)
            load_x(5)
            load_x(6)
            load_wg_g(1, 0)
            load_x(7)
            load_wg_g(1, 1)
            load_wg_g(1, 2)
            load_win(0)
            load_wres(0)
            # prev splits across both HWDGE rings (scalar is idle after x0),
            # so tiles 2-4's weights on sync aren't delayed behind it
            hb = BS // 2
            for k in range(KR):
                pk = acts.tile([128, BS], MMDT, tag=f"p{k}")
                eng = nc.scalar if k % 2 == 0 else nc.sync
                eng.dma_start(pk[:], p_d.ap()[k])
                p_ks.append(pk)

            ps_gates = {}

            def gate_mms(t, fillers=0):
                wg_t = wg_ts[t]
                ps_i = psum.tile([128, BS], F32, tag="ps_i")
                ps_f = psum.tile([128, BS], F32, tag="ps_f")
                ps_o = psum.tile([128, BS], F32, tag="ps_o")
                ps_gates[t] = (ps_i, ps_f, ps_o)
                # Filler matmuls on resident data keep the PE activity
                # monitor warm through DMA-starved stretches; the real
                # start=True matmul below discards their result.
                for j in range(fillers):
                    nc.tensor.matmul(ps_i[:, :FILLER_N], x_ks[0][:, :128],
                                     x_ks[1][:, :FILLER_N],
                                     start=(j == 0), stop=(j == fillers - 1))
                # k-outer: 3 matmuls per x chunk, so the cold-rate PE never
                # outruns the arriving x stream on the first tiles (keeps
                # the HAM activity window contiguous -> warms in 3.4us)
                for k in range(KD):
                    for ps, wslice in ((ps_i, wg_t[:, 0, k]),
                                       (ps_f, wg_t[:, 1, k]),
                                       (ps_o, wg_t[:, 2, k])):
                        nc.tensor.matmul(ps[:], wslice, x_ks[k][:],
                                         start=(k == 0), stop=(k == KD - 1))

            def epilogue(t, ps_s, lo, n):
                """state = o*(0.9*f*prev + 0.1*tanh(i*s)) + spike, columns
                [lo, lo+n)."""
                ps_i, ps_f, ps_o = ps_gates[t]
                sl = np.s_[:, lo:lo + n]
                prev_t = p_ks[t][sl]
                if MM_MODE == 'f32r':
                    prev_t = prev_t.bitcast(F32)
                si = epi.tile([128, BS], F32, tag="si")
                nc.scalar.activation(si[sl], ps_i[sl], AF.Sigmoid)
                sf = epi.tile([128, BS], F32, tag="sf")
                nc.scalar.activation(sf[sl], ps_f[sl], AF.Sigmoid)
                so = epi.tile([128, BS], F32, tag="so")
                nc.scalar.activation(so[sl], ps_o[sl], AF.Sigmoid)
                fp9 = epi.tile([128, BS], F32, tag="fp9")
                nc.vector.scalar_tensor_tensor(fp9[sl], sf[sl], 1.0 - LEAK,
                                               prev_t, OP.mult, OP.mult)
                x1 = epi.tile([128, BS], F32, tag="x1")
                nc.vector.tensor_tensor(x1[sl], si[sl], ps_s[sl], OP.mult)
                th = epi.tile([128, BS], F32, tag="th")
                nc.scalar.activation(th[sl], x1[sl], AF.Tanh)
                pre = epi.tile([128, BS], F32, tag="pre")
                nc.vector.scalar_tensor_tensor(pre[sl], th[sl], LEAK, fp9[sl],
                                               OP.mult, OP.add)
                st = epi.tile([128, BS], F32, tag="st")
                nc.vector.tensor_tensor(st[sl], pre[sl], so[sl], OP.mult)
                msk = epi.tile([128, BS], F32, tag="msk")
                nc.vector.tensor_scalar(msk[sl], st[sl], THRESH, THRESH,
                                        OP.is_gt, OP.mult)
                ot = epi.tile([128, BS], F32, tag="ot")
                nc.vector.tensor_tensor(ot[sl], st[sl], msk[sl], OP.subtract)
                # stores ride the otherwise-idle SWDGE ring so they never
                # contend with the input streams; the last tile's go on the
                # (by then idle) sync ring for low latency
                dma_eng = nc.sync if t == RT - 1 else nc.gpsimd
                dma_eng.dma_start(out_d.ap()[t * 128:(t + 1) * 128, lo:lo + n],
                                  ot[sl])

            # ---- pipelined main loop: gates for t+1, then state GEMM +
            # epilogue for t.
            gate_mms(0)
            for t in range(RT):
                # prefetch loads, in next-use order
                if t + 2 < RT:
                    for g in range(3):
                        load_wg_g(t + 2, g)
                if t + 1 < RT:
                    load_win(t + 1)
                    load_wres(t + 1)

                if t + 1 < RT:
                    gate_mms(t + 1)

                # s = input_part + reservoir_part for tile t
                win_t, wres_t = win_ts.pop(t), wres_ts.pop(t)
                ps_s = psum.tile([128, BS], F32, tag="ps_s")
                if t == RT - 1:
                    # last tile: column-halved GEMM (N=256 keeps the full
                    # fp32r rate) so the left half's epilogue overlaps the
                    # right half's matmuls and the tail chain is short
                    for lo in (0, hb):
                        cs = np.s_[:, lo:lo + hb]
                        for k in range(KD):
                            nc.tensor.matmul(ps_s[cs], win_t[:, k],
                                             x_ks[k][cs],
                                             start=(k == 0), stop=False)
                        for k in range(KR):
                            nc.tensor.matmul(ps_s[cs], wres_t[:, k],
                                             p_ks[k][cs],
                                             start=False, stop=(k == KR - 1))
                        epilogue(t, ps_s, lo, hb)
                else:
                    for k in range(KD):
                        nc.tensor.matmul(ps_s[:], win_t[:, k], x_ks[k][:],
                                         start=(k == 0), stop=False)
                    for k in range(KR):
                        nc.tensor.matmul(ps_s[:], wres_t[:, k], p_ks[k][:],
                                         start=False, stop=(k == KR - 1))
                    epilogue(t, ps_s, 0, BS)
                del ps_gates[t]

    nc.compile()
    return nc


def _get_nc():
    if 'nc' not in _cache:
        _cache['nc'] = _build_nc()
    return _cache['nc']


def _pack_inputs(inputs, prev_state, W_in, W_res, W_gate):
    """Host-side packing: transpose so contraction dim lands on SBUF
    partitions, with per-partition-contiguous DMA blocks."""
    f = np.float32
    # x[c, k, p, b] = inputs[512c + b, 128k + p]
    xp = np.ascontiguousarray(
        inputs.reshape(N_CORES, BS, KD, 128).transpose(0, 2, 3, 1).astype(f, copy=False))
    # p[c, k, p, b] = prev_state[512c + b, 128k + p]
    pp = np.ascontiguousarray(
        prev_state[:, :R].reshape(N_CORES, BS, KR, 128).transpose(0, 2, 3, 1).astype(f, copy=False))
    # win[t, p, k, m] = W_in[128t + m, 128k + p]
    win = np.ascontiguousarray(
        W_in.reshape(RT, 128, KD, 128).transpose(0, 3, 2, 1).astype(f, copy=False))
    # wres[t, p, j, m] = W_res[128t + m, 128j + p]
    wres = np.ascontiguousarray(
        W_res.reshape(RT, 128, KR, 128).transpose(0, 3, 2, 1).astype(f, copy=False))
    # wg[g, t, p, k, m] = W_gate[2048g + 128t + m, 128k + p]
    wg = np.ascontiguousarray(
        W_gate.reshape(3, RT, 128, KD, 128).transpose(0, 1, 4, 3, 2).astype(f, copy=False))

    in_maps = []
    for c in range(N_CORES):
        in_maps.append({
            "x": xp[c], "p": pp[c],
            "win": win, "wres": wres, "wg": wg,
        })
    return in_maps


def _assemble(results):
    out = np.zeros((B, MAX_DIM), dtype=np.float32)
    for c in range(N_CORES):
        out[c * BS:(c + 1) * BS, :R] = results[c]["out"].T
    return out


def _run(in_maps, **spmd_kwargs):
    from concourse.bass_utils import run_bass_kernel_spmd
    nc = _get_nc()
    return run_bass_kernel_spmd(nc, in_maps, core_ids=list(range(N_CORES)),
                                **spmd_kwargs)


def _device_ok():
    """True if this process can reach the 8 trn2 NeuronCores via PJRT."""
    try:
        import jax
        return len([d for d in jax.devices() if 'NC' in str(d)
                    or d.platform not in ('cpu',)]) >= N_CORES
    except Exception:
        return False


def _kernel_subprocess(inputs, prev_state, W_in, W_res, W_gate):
    """Run in a child process with a clean JAX_PLATFORMS so the axon PJRT
    backend is reachable even if the caller pinned jax to cpu."""
    import os
    import subprocess
    import sys
    import tempfile
    tmp = tempfile.mkdtemp(prefix="gsr_kernel_")
    in_npz = os.path.join(tmp, "in.npz")
    out_npy = os.path.join(tmp, "out.npy")
    np.savez(in_npz, inputs=np.asarray(inputs),
             prev_state=np.asarray(prev_state), W_in=np.asarray(W_in),
             W_res=np.asarray(W_res), W_gate=np.asarray(W_gate))
    code = (
        "import sys, numpy as np\n"
        f"sys.path.insert(0, {os.path.dirname(os.path.abspath(__file__))!r})\n"
        "import kernel\n"
        f"d = np.load({in_npz!r})\n"
        "out = kernel._kernel_impl(**{k: d[k] for k in d.files})\n"
        f"np.save({out_npy!r}, out)\n"
    )
    env = dict(os.environ)
    env.pop("JAX_PLATFORMS", None)
    for attempt in range(3):
        r = subprocess.run([sys.executable, "-c", code], env=env)
        if r.returncode == 0:
            return np.load(out_npy)
    r.check_returncode()


def _kernel_impl(inputs, prev_state, W_in, W_res, W_gate):
    in_maps = _pack_inputs(np.asarray(inputs), np.asarray(prev_state),
                           np.asarray(W_in), np.asarray(W_res),
                           np.asarray(W_gate))
    res = _run(in_maps)
    return _assemble(res.results)


def kernel(inputs, prev_state, W_in, W_res, W_gate):
    if _device_ok():
        try:
            return _kernel_impl(inputs, prev_state, W_in, W_res, W_gate)
        except Exception:
            # transient device errors (e.g. NRT_EXEC_UNIT_UNRECOVERABLE)
            # recover in a fresh process with a fresh PJRT client
            pass
    return _kernel_subprocess(inputs, prev_state, W_in, W_res, W_gate)
